# revision 67
# baseline (speedup 1.0000x reference)
"""Trainium2 kernel for nn_KernelEncodingLayer (von Mises kernel encoding).

Math
----
reference computes, per key n and bin b:
    logits[n,b] = sum_f mag[n,f] * sum_k w[b,f,k] * exp(kappa*(cos(angle[n,f]-mu_eff[b,f,k])-1))

The von Mises kernel expands exactly in a Fourier series (Bessel coefficients):
    exp(kappa*cos(d))*exp(-kappa) = e^-kappa * [I_0(kappa) + 2*sum_m I_m(kappa) cos(m d)]
kappa <= 1 so the series converges superexponentially; truncating cos at m<=2
and sin at m<=3 leaves ~7e-3 max relative error (gate is 2e-2).

With r = mag, u = cos(angle) = x/r, y = r*sin(angle), the needed features are
p_j = r*u^j and q_j = y*u^j, folded with host-side Chebyshev/Bessel math into
per-(bin,freq) weights.  Device chunk layout (contraction rows = 128
partitions; top 64 = p-feature per freq, bottom 64 = q-feature per freq):
    chunk0 = [x ; y  ]  -> (P1, Q0)
    chunk1 = [xu; yu ]  -> (P2, Q1)
    chunk2 = [r ; yu2]  -> (P0, Q2)

Device kernel (per core, 1024 keys, two 512-key blocks pipelined), fp16 on the
wire and fp32 PSUM.  Host ships XY=[x;y], XX=[x;x], YY=[y;y] so every chain op
is a partition-uniform elementwise op with no cross-partition copies:
    sq=XX*XX (V / Pool)      syb=YY*YY (A square)     r2=sq+syb (V)
    rf32=sqrt(r2+eps) (A)    ir16=~1/rf32 (V custom-DVE approx recip, fp16 out)
    xyx=XY*XX=[x^2;xy] (V, early)   W1=xyx*ir16=[xu;yu] (V)
    W2.top=sqrt(r2.top) (A)         W2.bot=W1.bot*(x*ir16).bot (V)
then a [128 x 512] @ [128 x 128bins] fp16 PE matmul per chunk accumulated in
PSUM (one bank per key block), evicted fp16 via ACT (bias is added on host)
and DMA'd out per block.  A tiny ACT sqrt pre-warms the activation table
during the DMA fill.

Hard-won constraints baked in here (measured on HW, not guesses):
  - DVE/Pool ops must have dtype-uniform INPUTS: mixed fp16xfp32
    tensor_tensor falls off a microcode cliff (~11 ns/elem vs ~0.8).
  - nc.vector.reciprocal is ~6 cycles/elem; reciprocal_approx_fast is 1 —
    and its fp32-out assert can be bypassed (fp16 out converts on write).
  - GPSIMD cannot touch PSUM and must not convert dtypes.
  - Only ~23 us of a 37 us baseline is controllable: ~8.5 us is a fixed
    NEFF teardown (253 per-semaphore resets) and ~4 us preamble+DMA fill.

Sharding: data-parallel over keys across 8 cores; weights replicated.
"""

import math

import numpy as np

import concourse.bacc as bacc
import concourse.bass as bass
import concourse.mybir as mybir
import concourse.tile as tile
from concourse._compat import with_exitstack
from concourse.bass_utils import run_bass_kernel_spmd
from concourse.mybir import AluOpType

# problem shape (hardcoded per harness contract)
NKEYS = 8192
NBINS = 128
NFREQ = 64
NCORES = 8
KPC = NKEYS // NCORES  # 1024 keys per core
NCHUNK = 3  # contraction chunks: cos harmonics m<=2, sin m<=3
NSPLIT = 2  # key blocks per core for pipelining (PSUM bank per block)
BLK = KPC // NSPLIT

F16 = mybir.dt.float16
F32 = mybir.dt.float32
EPS_GUARD = 1e-6  # r2 guard so 1/r stays bounded

AFT = mybir.ActivationFunctionType


# ----------------------------------------------------------------------------
# host-side math: Bessel I_m and Chebyshev coefficient folding
# ----------------------------------------------------------------------------

def _bessel_i(m: int, x: np.ndarray) -> np.ndarray:
    x = np.asarray(x, np.float64)
    s = np.zeros_like(x)
    for j in range(24):
        s = s + (x / 2.0) ** (2 * j + m) / (math.factorial(j) * math.factorial(j + m))
    return s


def _cheb_t(m: int) -> np.ndarray:
    T = [np.array([1.0]), np.array([0.0, 1.0])]
    while len(T) <= m:
        a = np.zeros(len(T[-1]) + 1)
        a[1:] = 2 * T[-1]
        a[: len(T[-2])] -= T[-2]
        T.append(a)
    return T[m]


def _cheb_u(m: int) -> np.ndarray:
    U = [np.array([1.0]), np.array([0.0, 2.0])]
    while len(U) <= m:
        a = np.zeros(len(U[-1]) + 1)
        a[1:] = 2 * U[-1]
        a[: len(U[-2])] -= U[-2]
        U.append(a)
    return U[m]


def _build_device_weights(reference_angles, mu, kappa, weight) -> np.ndarray:
    """Fold per-(bin,freq) coefficients into [128, NCHUNK*NBINS] fp16.

    Column block c holds chunk c's weights; rows 0:64 multiply the p-feature,
    rows 64:128 the q-feature of that chunk.
    """
    mc, ms = 2, 2  # cos harmonics m<=mc, sin m<=ms
    mu_eff = np.asarray(mu, np.float64) + np.asarray(reference_angles, np.float64)[None, :, None]
    kap = np.asarray(kappa, np.float64)
    w = np.asarray(weight, np.float64)

    P = np.zeros((mc + 1, NBINS, NFREQ))  # coeff of p_j = r*u^j
    Q = np.zeros((ms, NBINS, NFREQ))      # coeff of q_j = y*u^j
    for m in range(0, mc + 1):
        eps = 1.0 if m == 0 else 2.0
        coef = w * eps * _bessel_i(m, kap) * np.exp(-kap)
        A = (coef * np.cos(m * mu_eff)).sum(-1)  # (b, f)
        for j, c in enumerate(_cheb_t(m)):
            if c:
                P[j] += c * A
    for m in range(1, ms + 1):
        coef = w * 2.0 * _bessel_i(m, kap) * np.exp(-kap)
        B = (coef * np.sin(m * mu_eff)).sum(-1)
        for j, c in enumerate(_cheb_u(m - 1)):
            if c:
                Q[j] += c * B

    W = np.zeros((128, NCHUNK * NBINS), np.float64)
    pairs = [(P[1], Q[0]), (P[2], Q[1]), (P[0], None)]
    for c, (top, bot) in enumerate(pairs):
        W[:NFREQ, c * NBINS:(c + 1) * NBINS] = top.T  # (f, b)
        if bot is not None:
            W[NFREQ:, c * NBINS:(c + 1) * NBINS] = bot.T
    return np.ascontiguousarray(W.astype(np.float16))


# ----------------------------------------------------------------------------
# device kernel
# ----------------------------------------------------------------------------

@with_exitstack
def _device_kernel(ctx, tc: tile.TileContext, out_d, pk_d, w_d):
    nc = tc.nc
    const = ctx.enter_context(tc.tile_pool(name="const", bufs=1))
    work = ctx.enter_context(tc.tile_pool(name="work", bufs=1))
    psum = ctx.enter_context(tc.tile_pool(name="psum", bufs=1, space="PSUM"))

    # eps doubles as the r2 guard bias for sqrt and as the operand of a tiny
    # warm-up op that pulls the ACT table load into the DMA-fill window
    eps = const.tile([128, 1], F32, tag="eps")
    warm = const.tile([128, 1], F32, tag="warm")
    nc.gpsimd.memset(eps[:], EPS_GUARD)
    nc.scalar.sqrt(warm[:], eps[:])

    # pk packs [yy-h | xx-h | xy-h] per key half: one DMA brings everything
    # the half's chain needs, so only 3 serialized issues instead of 7 and
    # h1's inputs land ~0.7us earlier
    pk = const.tile([128, 3 * KPC], F16, tag="pk")
    wt = const.tile([128, NCHUNK * NBINS], F16, tag="wt")
    nc.sync.dma_start(pk[:, :3 * BLK], pk_d[:, :3 * BLK])
    nc.sync.dma_start(pk[:, 3 * BLK:], pk_d[:, 3 * BLK:])
    nc.sync.dma_start(wt[:], w_d[:])

    def yyb(h):
        return pk[:, h * 3 * BLK: h * 3 * BLK + BLK]

    def xxb(h):
        return pk[:, h * 3 * BLK + BLK: h * 3 * BLK + 2 * BLK]

    def xyb(h):
        return pk[:, h * 3 * BLK + 2 * BLK: h * 3 * BLK + 3 * BLK]

    sq = work.tile([128, KPC], F16, tag="sq")
    syb = work.tile([128, KPC], F16, tag="syb")
    r2 = work.tile([128, KPC], F16, tag="r2")
    rf32 = work.tile([128, KPC], F32, tag="rf32")
    ir16 = work.tile([128, KPC], F16, tag="ir16")
    xyx = work.tile([128, KPC], F16, tag="xyx")
    uf = work.tile([128, KPC], F16, tag="uf")
    w1 = work.tile([128, KPC], F16, tag="w1")
    w2 = work.tile([128, KPC], F16, tag="w2")
    outt = work.tile([128, KPC], F16, tag="outt")

    HF = NFREQ
    ps = [psum.tile([128, BLK], F32, tag=f"ps{h}", name=f"ps{h}") for h in range(NSPLIT)]

    def blk(t, h):
        return t[:, h * BLK:(h + 1) * BLK]

    def blkb(t, h):  # bottom half of a block
        return t[HF:, h * BLK:(h + 1) * BLK]

    def blkt(t, h):  # top half of a block
        return t[:HF, h * BLK:(h + 1) * BLK]

    def recip_fast_f16(out, in_):
        # reciprocal_approx_fast with an fp16 output AP: the BITWISE_NOT seed
        # only needs the fp32 INPUT bit layout; the write-side converts.
        from concourse.dve_ops import RECIP_APPROX_FAST_CONSTS, RECIPROCAL_APPROX_FAST
        c = RECIP_APPROX_FAST_CONSTS
        return nc.vector._custom_dve(
            RECIPROCAL_APPROX_FAST, out=out, in0=in_,
            s0=c["s0"], s1=c["s1"], imm2=c["imm2"],
        )

    import contextlib

    with nc.allow_low_precision(reason="fp16 feature chain; validated vs fp64 host sim"):
        for h in range(NSPLIT):
            # h0's chain outranks h1 in the scheduler so engine queues don't
            # stall behind not-yet-ready h1 ops
            with tc.high_priority() if h == 0 else contextlib.nullcontext():
                # chunk0 matmul only needs xy + weights; runs during the chain
                nc.tensor.matmul(ps[h][:], wt[:, 0:NBINS], xyb(h), start=True, stop=False)

                # squares: h0's x^2 on V (spine head), h1's on Pool
                if h == 0:
                    nc.vector.tensor_tensor(blk(sq, h), xxb(h), xxb(h), AluOpType.mult)
                else:
                    nc.gpsimd.tensor_tensor(blk(sq, h), xxb(h), xxb(h), AluOpType.mult)
                nc.scalar.square(blk(syb, h), yyb(h))
                nc.vector.tensor_tensor(blk(r2, h), blk(sq, h), blk(syb, h), AluOpType.add)
                nc.scalar.activation(blk(rf32, h), blk(r2, h), AFT.Sqrt, bias=eps[:])
                recip_fast_f16(blk(ir16, h), blk(rf32, h))
                # chunk2: just the r feature on top; the bottom 64 contraction
                # rows are dropped entirely (sin series truncated at m<=2), so
                # this is a 64-partition matmul ready straight off the sqrt,
                # emitted mid-group so the stop lands on the late w1 matmul.
                nc.scalar.activation(blkt(w2, h), blkt(r2, h), AFT.Sqrt, bias=eps[:HF])
                nc.tensor.matmul(ps[h][:], wt[:HF, 2 * NBINS:3 * NBINS],
                                 w2[:HF, h * BLK:(h + 1) * BLK], start=False, stop=False)
                # xyx = [x^2; xy] is input-only; scheduled into early V gaps.
                # w1 = xyx*(1/r) = [xu; yu] comes straight off the recip.
                nc.vector.tensor_tensor(blk(xyx, h), xyb(h), xxb(h), AluOpType.mult)
                nc.vector.tensor_tensor(blk(w1, h), blk(xyx, h), blk(ir16, h), AluOpType.mult)
                nc.tensor.matmul(ps[h][:], wt[:, NBINS:2 * NBINS], blk(w1, h), start=False, stop=True)

                # evict PSUM -> SBUF fp16 (bias added on host), then DMA out
                nc.scalar.copy(blk(outt, h), ps[h][:])
                nc.scalar.dma_start(out_d[:, h * BLK:(h + 1) * BLK], blk(outt, h))


_COMPILED = None


def _get_compiled():
    global _COMPILED
    if _COMPILED is None:
        nc = bacc.Bacc("TRN2", target_bir_lowering=False, debug=False)
        pk = nc.dram_tensor("pk", [128, 3 * KPC], F16, kind="ExternalInput").ap()
        w = nc.dram_tensor("w", [128, NCHUNK * NBINS], F16, kind="ExternalInput").ap()
        out = nc.dram_tensor("out", [NBINS, KPC], F16, kind="ExternalOutput").ap()
        with tile.TileContext(nc) as tc:
            _device_kernel(tc, out, pk, w)
        nc.compile()
        _COMPILED = nc
    return _COMPILED


# ----------------------------------------------------------------------------
# entry point
# ----------------------------------------------------------------------------

def _run(K, reference_angles, mu, kappa, weight, bias, **spmd_kwargs):
    K = np.ascontiguousarray(np.asarray(K, np.float32))
    x = K[:, 0::2].astype(np.float16)  # (NKEYS, NFREQ) real parts
    y = K[:, 1::2].astype(np.float16)  # imag parts

    W = _build_device_weights(reference_angles, mu, kappa, weight)
    in_maps = []
    for c in range(NCORES):
        sl = slice(c * KPC, (c + 1) * KPC)
        xt = np.ascontiguousarray(x[sl].T)  # (64, KPC)
        yt = np.ascontiguousarray(y[sl].T)
        # pk packs [yy-h | xx-h | xy-h] per key half (replicated x/y halves)
        pk = np.empty((128, 3 * KPC), np.float16)
        for h in range(NSPLIT):
            ksl = slice(h * BLK, (h + 1) * BLK)
            base = h * 3 * BLK
            pk[:NFREQ, base:base + BLK] = yt[:, ksl]
            pk[NFREQ:, base:base + BLK] = yt[:, ksl]
            pk[:NFREQ, base + BLK:base + 2 * BLK] = xt[:, ksl]
            pk[NFREQ:, base + BLK:base + 2 * BLK] = xt[:, ksl]
            pk[:NFREQ, base + 2 * BLK:base + 3 * BLK] = xt[:, ksl]
            pk[NFREQ:, base + 2 * BLK:base + 3 * BLK] = yt[:, ksl]
        in_maps.append({"pk": pk, "w": W})

    nc = _get_compiled()
    res = run_bass_kernel_spmd(nc, in_maps, list(range(NCORES)), **spmd_kwargs)

    bias32 = np.asarray(bias, np.float32)
    out = np.empty((NKEYS, NBINS), np.float32)
    for c in range(NCORES):
        out[c * KPC:(c + 1) * KPC] = res.results[c]["out"].T.astype(np.float32)
    out += bias32[None, :]
    return out, res


def kernel(K, reference_angles, mu, kappa, weight, bias):
    out, _ = _run(K, reference_angles, mu, kappa, weight, bias)
    return out


# revision 69
# speedup vs baseline: 1.0697x; 1.0697x over previous
"""Trainium2 kernel for nn_KernelEncodingLayer (von Mises kernel encoding).

Math
----
reference computes, per key n and bin b:
    logits[n,b] = sum_f mag[n,f] * sum_k w[b,f,k] * exp(kappa*(cos(angle[n,f]-mu_eff[b,f,k])-1))

The von Mises kernel expands exactly in a Fourier series (Bessel coefficients):
    exp(kappa*cos(d))*exp(-kappa) = e^-kappa * [I_0(kappa) + 2*sum_m I_m(kappa) cos(m d)]
kappa <= 1 so the series converges superexponentially; truncating cos at m<=2
and sin at m<=3 leaves ~7e-3 max relative error (gate is 2e-2).

With r = mag, u = cos(angle) = x/r, y = r*sin(angle), the needed features are
p_j = r*u^j and q_j = y*u^j, folded with host-side Chebyshev/Bessel math into
per-(bin,freq) weights.  Device chunk layout (contraction rows = 128
partitions; top 64 = p-feature per freq, bottom 64 = q-feature per freq):
    chunk0 = [x ; y  ]  -> (P1, Q0)
    chunk1 = [xu; yu ]  -> (P2, Q1)
    chunk2 = [r ; yu2]  -> (P0, Q2)

Device kernel (per core, 1024 keys, two 512-key blocks pipelined), fp16 on the
wire and fp32 PSUM.  Host ships XY=[x;y], XX=[x;x], YY=[y;y] so every chain op
is a partition-uniform elementwise op with no cross-partition copies:
    sq=XX*XX (V / Pool)      syb=YY*YY (A square)     r2=sq+syb (V)
    rf32=sqrt(r2+eps) (A)    ir16=~1/rf32 (V custom-DVE approx recip, fp16 out)
    xyx=XY*XX=[x^2;xy] (V, early)   W1=xyx*ir16=[xu;yu] (V)
    W2.top=sqrt(r2.top) (A)         W2.bot=W1.bot*(x*ir16).bot (V)
then a [128 x 512] @ [128 x 128bins] fp16 PE matmul per chunk accumulated in
PSUM (one bank per key block), evicted fp16 via ACT (bias is added on host)
and DMA'd out per block.  A tiny ACT sqrt pre-warms the activation table
during the DMA fill.

Hard-won constraints baked in here (measured on HW, not guesses):
  - DVE/Pool ops must have dtype-uniform INPUTS: mixed fp16xfp32
    tensor_tensor falls off a microcode cliff (~11 ns/elem vs ~0.8).
  - nc.vector.reciprocal is ~6 cycles/elem; reciprocal_approx_fast is 1 —
    and its fp32-out assert can be bypassed (fp16 out converts on write).
  - GPSIMD cannot touch PSUM and must not convert dtypes.
  - Only ~23 us of a 37 us baseline is controllable: ~8.5 us is a fixed
    NEFF teardown (253 per-semaphore resets) and ~4 us preamble+DMA fill.

Sharding: data-parallel over keys across 8 cores; weights replicated.
"""

import math

import numpy as np

import concourse.bacc as bacc
import concourse.bass as bass
import concourse.mybir as mybir
import concourse.tile as tile
from concourse._compat import with_exitstack
from concourse.bass_utils import run_bass_kernel_spmd
from concourse.mybir import AluOpType

# problem shape (hardcoded per harness contract)
NKEYS = 8192
NBINS = 128
NFREQ = 64
NCORES = 8
KPC = NKEYS // NCORES  # 1024 keys per core
NCHUNK = 3  # contraction chunks: cos harmonics m<=2, sin m<=3
NSPLIT = 2  # key blocks per core for pipelining (PSUM bank per block)
BLK = KPC // NSPLIT

F16 = mybir.dt.float16
F32 = mybir.dt.float32
EPS_GUARD = 1e-6  # r2 guard so 1/r stays bounded

AFT = mybir.ActivationFunctionType


# ----------------------------------------------------------------------------
# host-side math: Bessel I_m and Chebyshev coefficient folding
# ----------------------------------------------------------------------------

def _bessel_i(m: int, x: np.ndarray) -> np.ndarray:
    x = np.asarray(x, np.float64)
    s = np.zeros_like(x)
    for j in range(24):
        s = s + (x / 2.0) ** (2 * j + m) / (math.factorial(j) * math.factorial(j + m))
    return s


def _cheb_t(m: int) -> np.ndarray:
    T = [np.array([1.0]), np.array([0.0, 1.0])]
    while len(T) <= m:
        a = np.zeros(len(T[-1]) + 1)
        a[1:] = 2 * T[-1]
        a[: len(T[-2])] -= T[-2]
        T.append(a)
    return T[m]


def _cheb_u(m: int) -> np.ndarray:
    U = [np.array([1.0]), np.array([0.0, 2.0])]
    while len(U) <= m:
        a = np.zeros(len(U[-1]) + 1)
        a[1:] = 2 * U[-1]
        a[: len(U[-2])] -= U[-2]
        U.append(a)
    return U[m]


def _build_device_weights(reference_angles, mu, kappa, weight) -> np.ndarray:
    """Fold per-(bin,freq) coefficients into [128, NCHUNK*NBINS] fp16.

    Column block c holds chunk c's weights; rows 0:64 multiply the p-feature,
    rows 64:128 the q-feature of that chunk.
    """
    mc, ms = 2, 2  # cos harmonics m<=mc, sin m<=ms
    mu_eff = np.asarray(mu, np.float64) + np.asarray(reference_angles, np.float64)[None, :, None]
    kap = np.asarray(kappa, np.float64)
    w = np.asarray(weight, np.float64)

    P = np.zeros((mc + 1, NBINS, NFREQ))  # coeff of p_j = r*u^j
    Q = np.zeros((ms, NBINS, NFREQ))      # coeff of q_j = y*u^j
    for m in range(0, mc + 1):
        eps = 1.0 if m == 0 else 2.0
        coef = w * eps * _bessel_i(m, kap) * np.exp(-kap)
        A = (coef * np.cos(m * mu_eff)).sum(-1)  # (b, f)
        for j, c in enumerate(_cheb_t(m)):
            if c:
                P[j] += c * A
    for m in range(1, ms + 1):
        coef = w * 2.0 * _bessel_i(m, kap) * np.exp(-kap)
        B = (coef * np.sin(m * mu_eff)).sum(-1)
        for j, c in enumerate(_cheb_u(m - 1)):
            if c:
                Q[j] += c * B

    W = np.zeros((128, NCHUNK * NBINS), np.float64)
    pairs = [(P[1], Q[0]), (P[2], Q[1]), (P[0], None)]
    for c, (top, bot) in enumerate(pairs):
        W[:NFREQ, c * NBINS:(c + 1) * NBINS] = top.T  # (f, b)
        if bot is not None:
            W[NFREQ:, c * NBINS:(c + 1) * NBINS] = bot.T
    return np.ascontiguousarray(W.astype(np.float16))


# ----------------------------------------------------------------------------
# device kernel
# ----------------------------------------------------------------------------

@with_exitstack
def _device_kernel(ctx, tc: tile.TileContext, out_d, pk_d, w_d):
    nc = tc.nc
    const = ctx.enter_context(tc.tile_pool(name="const", bufs=1))
    work = ctx.enter_context(tc.tile_pool(name="work", bufs=1))
    psum = ctx.enter_context(tc.tile_pool(name="psum", bufs=1, space="PSUM"))

    # eps doubles as the r2 guard bias for sqrt and as the operand of a tiny
    # warm-up op that pulls the ACT table load into the DMA-fill window
    eps = const.tile([128, 1], F32, tag="eps")
    warm = const.tile([128, 1], F32, tag="warm")
    nc.gpsimd.memset(eps[:], EPS_GUARD)
    nc.scalar.sqrt(warm[:], eps[:])

    # pk packs [yy-h | xx-h | xy-h] per key half: one DMA brings everything
    # the half's chain needs, so only 3 serialized issues instead of 7 and
    # h1's inputs land ~0.7us earlier
    pk = const.tile([128, 3 * KPC], F16, tag="pk")
    wt = const.tile([128, NCHUNK * NBINS], F16, tag="wt")
    nc.sync.dma_start(pk[:, :3 * BLK], pk_d[:, :3 * BLK])
    nc.sync.dma_start(pk[:, 3 * BLK:], pk_d[:, 3 * BLK:])
    nc.sync.dma_start(wt[:], w_d[:])

    def yyb(h):
        return pk[:, h * 3 * BLK: h * 3 * BLK + BLK]

    def xxb(h):
        return pk[:, h * 3 * BLK + BLK: h * 3 * BLK + 2 * BLK]

    def xyb(h):
        return pk[:, h * 3 * BLK + 2 * BLK: h * 3 * BLK + 3 * BLK]

    sq = work.tile([128, KPC], F16, tag="sq")
    syb = work.tile([128, KPC], F16, tag="syb")
    r2 = work.tile([128, KPC], F16, tag="r2")
    rf32 = work.tile([128, KPC], F32, tag="rf32")
    ir16 = work.tile([128, KPC], F16, tag="ir16")
    xyx = work.tile([128, KPC], F16, tag="xyx")
    uf = work.tile([128, KPC], F16, tag="uf")
    w1 = work.tile([128, KPC], F16, tag="w1")
    w2 = work.tile([128, KPC], F16, tag="w2")
    outt = work.tile([128, KPC], F16, tag="outt")

    HF = NFREQ
    ps = [psum.tile([128, BLK], F32, tag=f"ps{h}", name=f"ps{h}") for h in range(NSPLIT)]

    def blk(t, h):
        return t[:, h * BLK:(h + 1) * BLK]

    def blkb(t, h):  # bottom half of a block
        return t[HF:, h * BLK:(h + 1) * BLK]

    def blkt(t, h):  # top half of a block
        return t[:HF, h * BLK:(h + 1) * BLK]

    def recip_fast_f16(out, in_):
        # reciprocal_approx_fast with an fp16 output AP: the BITWISE_NOT seed
        # only needs the fp32 INPUT bit layout; the write-side converts.
        from concourse.dve_ops import RECIP_APPROX_FAST_CONSTS, RECIPROCAL_APPROX_FAST
        c = RECIP_APPROX_FAST_CONSTS
        return nc.vector._custom_dve(
            RECIPROCAL_APPROX_FAST, out=out, in0=in_,
            s0=c["s0"], s1=c["s1"], imm2=c["imm2"],
        )

    import contextlib

    with nc.allow_low_precision(reason="fp16 feature chain; validated vs fp64 host sim"):
        for h in range(NSPLIT):
            # h0's chain outranks h1 in the scheduler so engine queues don't
            # stall behind not-yet-ready h1 ops
            with tc.high_priority() if h == 0 else contextlib.nullcontext():
                # chunk0 matmul only needs xy + weights; runs during the chain
                nc.tensor.matmul(ps[h][:], wt[:, 0:NBINS], xyb(h), start=True, stop=False)

                # squares: h0's x^2 on V (spine head), h1's on Pool
                if h == 0:
                    nc.vector.tensor_tensor(blk(sq, h), xxb(h), xxb(h), AluOpType.mult)
                else:
                    nc.gpsimd.tensor_tensor(blk(sq, h), xxb(h), xxb(h), AluOpType.mult)
                nc.scalar.square(blk(syb, h), yyb(h))
                nc.vector.tensor_tensor(blk(r2, h), blk(sq, h), blk(syb, h), AluOpType.add)
                nc.scalar.activation(blk(rf32, h), blk(r2, h), AFT.Sqrt, bias=eps[:])
                recip_fast_f16(blk(ir16, h), blk(rf32, h))
                # chunk2: just the r feature on top; the bottom 64 contraction
                # rows are dropped entirely (sin series truncated at m<=2), so
                # this is a 64-partition matmul ready straight off the sqrt,
                # emitted mid-group so the stop lands on the late w1 matmul.
                nc.scalar.activation(blkt(w2, h), blkt(r2, h), AFT.Sqrt, bias=eps[:HF])
                nc.tensor.matmul(ps[h][:], wt[:HF, 2 * NBINS:3 * NBINS],
                                 w2[:HF, h * BLK:(h + 1) * BLK], start=False, stop=False)
                # xyx = [x^2; xy] is input-only; scheduled into early V gaps.
                # w1 = xyx*(1/r) = [xu; yu] comes straight off the recip.
                nc.vector.tensor_tensor(blk(xyx, h), xyb(h), xxb(h), AluOpType.mult)
                nc.vector.tensor_tensor(blk(w1, h), blk(xyx, h), blk(ir16, h), AluOpType.mult)
                nc.tensor.matmul(ps[h][:], wt[:, NBINS:2 * NBINS], blk(w1, h), start=False, stop=True)

                # evict PSUM -> SBUF fp16 (bias added on host), then DMA out
                nc.scalar.copy(blk(outt, h), ps[h][:])
                nc.scalar.dma_start(out_d[:, h * BLK:(h + 1) * BLK], blk(outt, h))


_COMPILED = None


def _get_compiled():
    global _COMPILED
    if _COMPILED is None:
        nc = bacc.Bacc("TRN2", target_bir_lowering=False, debug=False)
        pk = nc.dram_tensor("pk", [128, 3 * KPC], F16, kind="ExternalInput").ap()
        w = nc.dram_tensor("w", [128, NCHUNK * NBINS], F16, kind="ExternalInput").ap()
        out = nc.dram_tensor("out", [NBINS, KPC], F16, kind="ExternalOutput").ap()
        with tile.TileContext(nc) as tc:
            _device_kernel(tc, out, pk, w)
        nc.compile()
        _COMPILED = nc
    return _COMPILED


# ----------------------------------------------------------------------------
# entry point
# ----------------------------------------------------------------------------

def _run(K, reference_angles, mu, kappa, weight, bias, **spmd_kwargs):
    K = np.ascontiguousarray(np.asarray(K, np.float32))
    x = K[:, 0::2].astype(np.float16)  # (NKEYS, NFREQ) real parts
    y = K[:, 1::2].astype(np.float16)  # imag parts

    W = _build_device_weights(reference_angles, mu, kappa, weight)
    in_maps = []
    for c in range(NCORES):
        sl = slice(c * KPC, (c + 1) * KPC)
        xt = np.ascontiguousarray(x[sl].T)  # (64, KPC)
        yt = np.ascontiguousarray(y[sl].T)
        # pk packs [yy-h | xx-h | xy-h] per key half (replicated x/y halves)
        pk = np.empty((128, 3 * KPC), np.float16)
        for h in range(NSPLIT):
            ksl = slice(h * BLK, (h + 1) * BLK)
            base = h * 3 * BLK
            pk[:NFREQ, base:base + BLK] = yt[:, ksl]
            pk[NFREQ:, base:base + BLK] = yt[:, ksl]
            pk[:NFREQ, base + BLK:base + 2 * BLK] = xt[:, ksl]
            pk[NFREQ:, base + BLK:base + 2 * BLK] = xt[:, ksl]
            pk[:NFREQ, base + 2 * BLK:base + 3 * BLK] = xt[:, ksl]
            pk[NFREQ:, base + 2 * BLK:base + 3 * BLK] = yt[:, ksl]
        in_maps.append({"pk": pk, "w": W})

    nc = _get_compiled()
    res = run_bass_kernel_spmd(nc, in_maps, list(range(NCORES)), **spmd_kwargs)

    bias32 = np.asarray(bias, np.float32)
    out = np.empty((NKEYS, NBINS), np.float32)
    for c in range(NCORES):
        out[c * KPC:(c + 1) * KPC] = res.results[c]["out"].T.astype(np.float32)
    out += bias32[None, :]
    return out, res


def kernel(K, reference_angles, mu, kappa, weight, bias):
    out, _ = _run(K, reference_angles, mu, kappa, weight, bias)
    return out


# revision 70
# speedup vs baseline: 1.1195x; 1.0465x over previous
"""Trainium2 kernel for nn_KernelEncodingLayer (von Mises kernel encoding).

Math
----
reference computes, per key n and bin b:
    logits[n,b] = sum_f mag[n,f] * sum_k w[b,f,k] * exp(kappa*(cos(angle[n,f]-mu_eff[b,f,k])-1))

The von Mises kernel expands exactly in a Fourier series (Bessel coefficients):
    exp(kappa*cos(d))*exp(-kappa) = e^-kappa * [I_0(kappa) + 2*sum_m I_m(kappa) cos(m d)]
kappa <= 1 so the series converges superexponentially; truncating cos at m<=2
and sin at m<=3 leaves ~7e-3 max relative error (gate is 2e-2).

With r = mag, u = cos(angle) = x/r, y = r*sin(angle), the needed features are
p_j = r*u^j and q_j = y*u^j, folded with host-side Chebyshev/Bessel math into
per-(bin,freq) weights.  Device chunk layout (contraction rows = 128
partitions; top 64 = p-feature per freq, bottom 64 = q-feature per freq):
    chunk0 = [x ; y  ]  -> (P1, Q0)
    chunk1 = [xu; yu ]  -> (P2, Q1)
    chunk2 = [r ; yu2]  -> (P0, Q2)

Device kernel (per core, 1024 keys, two 512-key blocks pipelined), fp16 on the
wire and fp32 PSUM.  Host ships XY=[x;y], XX=[x;x], YY=[y;y] so every chain op
is a partition-uniform elementwise op with no cross-partition copies:
    sq=XX*XX (V / Pool)      syb=YY*YY (A square)     r2=sq+syb (V)
    rf32=sqrt(r2+eps) (A)    ir16=~1/rf32 (V custom-DVE approx recip, fp16 out)
    xyx=XY*XX=[x^2;xy] (V, early)   W1=xyx*ir16=[xu;yu] (V)
    W2.top=sqrt(r2.top) (A)         W2.bot=W1.bot*(x*ir16).bot (V)
then a [128 x 512] @ [128 x 128bins] fp16 PE matmul per chunk accumulated in
PSUM (one bank per key block), evicted fp16 via ACT (bias is added on host)
and DMA'd out per block.  A tiny ACT sqrt pre-warms the activation table
during the DMA fill.

Hard-won constraints baked in here (measured on HW, not guesses):
  - DVE/Pool ops must have dtype-uniform INPUTS: mixed fp16xfp32
    tensor_tensor falls off a microcode cliff (~11 ns/elem vs ~0.8).
  - nc.vector.reciprocal is ~6 cycles/elem; reciprocal_approx_fast is 1 —
    and its fp32-out assert can be bypassed (fp16 out converts on write).
  - GPSIMD cannot touch PSUM and must not convert dtypes.
  - Only ~23 us of a 37 us baseline is controllable: ~8.5 us is a fixed
    NEFF teardown (253 per-semaphore resets) and ~4 us preamble+DMA fill.

Sharding: data-parallel over keys across 8 cores; weights replicated.
"""

import math

import numpy as np

import concourse.bacc as bacc
import concourse.bass as bass
import concourse.mybir as mybir
import concourse.tile as tile
from concourse._compat import with_exitstack
from concourse.bass_utils import run_bass_kernel_spmd
from concourse.mybir import AluOpType

# problem shape (hardcoded per harness contract)
NKEYS = 8192
NBINS = 128
NFREQ = 64
NCORES = 8
KPC = NKEYS // NCORES  # 1024 keys per core
NCHUNK = 3  # contraction chunks: cos harmonics m<=2, sin m<=3
NSPLIT = 2  # key blocks per core for pipelining (PSUM bank per block)
BLK = KPC // NSPLIT

F16 = mybir.dt.float16
F32 = mybir.dt.float32
EPS_GUARD = 1e-6  # r2 guard so 1/r stays bounded

AFT = mybir.ActivationFunctionType


# ----------------------------------------------------------------------------
# host-side math: Bessel I_m and Chebyshev coefficient folding
# ----------------------------------------------------------------------------

def _bessel_i(m: int, x: np.ndarray) -> np.ndarray:
    x = np.asarray(x, np.float64)
    s = np.zeros_like(x)
    for j in range(24):
        s = s + (x / 2.0) ** (2 * j + m) / (math.factorial(j) * math.factorial(j + m))
    return s


def _cheb_t(m: int) -> np.ndarray:
    T = [np.array([1.0]), np.array([0.0, 1.0])]
    while len(T) <= m:
        a = np.zeros(len(T[-1]) + 1)
        a[1:] = 2 * T[-1]
        a[: len(T[-2])] -= T[-2]
        T.append(a)
    return T[m]


def _cheb_u(m: int) -> np.ndarray:
    U = [np.array([1.0]), np.array([0.0, 2.0])]
    while len(U) <= m:
        a = np.zeros(len(U[-1]) + 1)
        a[1:] = 2 * U[-1]
        a[: len(U[-2])] -= U[-2]
        U.append(a)
    return U[m]


def _build_device_weights(reference_angles, mu, kappa, weight) -> np.ndarray:
    """Fold per-(bin,freq) coefficients into [128, NCHUNK*NBINS] fp16.

    Column block c holds chunk c's weights; rows 0:64 multiply the p-feature,
    rows 64:128 the q-feature of that chunk.
    """
    mc, ms = 2, 2  # cos harmonics m<=mc, sin m<=ms
    mu_eff = np.asarray(mu, np.float64) + np.asarray(reference_angles, np.float64)[None, :, None]
    kap = np.asarray(kappa, np.float64)
    w = np.asarray(weight, np.float64)

    P = np.zeros((mc + 1, NBINS, NFREQ))  # coeff of p_j = r*u^j
    Q = np.zeros((ms, NBINS, NFREQ))      # coeff of q_j = y*u^j
    for m in range(0, mc + 1):
        eps = 1.0 if m == 0 else 2.0
        coef = w * eps * _bessel_i(m, kap) * np.exp(-kap)
        A = (coef * np.cos(m * mu_eff)).sum(-1)  # (b, f)
        for j, c in enumerate(_cheb_t(m)):
            if c:
                P[j] += c * A
    for m in range(1, ms + 1):
        coef = w * 2.0 * _bessel_i(m, kap) * np.exp(-kap)
        B = (coef * np.sin(m * mu_eff)).sum(-1)
        for j, c in enumerate(_cheb_u(m - 1)):
            if c:
                Q[j] += c * B

    W = np.zeros((128, NCHUNK * NBINS), np.float64)
    pairs = [(P[1], Q[0]), (P[2], Q[1]), (P[0], None)]
    for c, (top, bot) in enumerate(pairs):
        W[:NFREQ, c * NBINS:(c + 1) * NBINS] = top.T  # (f, b)
        if bot is not None:
            W[NFREQ:, c * NBINS:(c + 1) * NBINS] = bot.T
    return np.ascontiguousarray(W.astype(np.float16))


# ----------------------------------------------------------------------------
# device kernel
# ----------------------------------------------------------------------------

@with_exitstack
def _device_kernel(ctx, tc: tile.TileContext, out_d, pk_d, w_d):
    nc = tc.nc
    const = ctx.enter_context(tc.tile_pool(name="const", bufs=1))
    work = ctx.enter_context(tc.tile_pool(name="work", bufs=1))
    psum = ctx.enter_context(tc.tile_pool(name="psum", bufs=1, space="PSUM"))

    # eps doubles as the r2 guard bias for sqrt and as the operand of a tiny
    # warm-up op that pulls the ACT table load into the DMA-fill window
    eps = const.tile([128, 1], F32, tag="eps")
    warm = const.tile([128, 1], F32, tag="warm")
    nc.gpsimd.memset(eps[:], EPS_GUARD)
    nc.scalar.sqrt(warm[:], eps[:])

    # pk packs [yy-h | xx-h | xy-h] per key half: one DMA brings everything
    # the half's chain needs, so only 3 serialized issues instead of 7 and
    # h1's inputs land ~0.7us earlier
    pk = const.tile([128, 3 * KPC], F16, tag="pk")
    wt = const.tile([128, NCHUNK * NBINS], F16, tag="wt")
    nc.sync.dma_start(pk[:, :3 * BLK], pk_d[:, :3 * BLK])
    nc.sync.dma_start(pk[:, 3 * BLK:], pk_d[:, 3 * BLK:])
    nc.sync.dma_start(wt[:], w_d[:])

    def yyb(h):
        return pk[:, h * 3 * BLK: h * 3 * BLK + BLK]

    def xxb(h):
        return pk[:, h * 3 * BLK + BLK: h * 3 * BLK + 2 * BLK]

    def xyb(h):
        return pk[:, h * 3 * BLK + 2 * BLK: h * 3 * BLK + 3 * BLK]

    sq = work.tile([128, KPC], F16, tag="sq")
    syb = work.tile([128, KPC], F16, tag="syb")
    r2 = work.tile([128, KPC], F16, tag="r2")
    rf32 = work.tile([128, KPC], F32, tag="rf32")
    ir16 = work.tile([128, KPC], F16, tag="ir16")
    xyx = work.tile([128, KPC], F16, tag="xyx")
    uf = work.tile([128, KPC], F16, tag="uf")
    w1 = work.tile([128, KPC], F16, tag="w1")
    w2 = work.tile([128, KPC], F16, tag="w2")
    outt = work.tile([128, KPC], F16, tag="outt")

    HF = NFREQ
    ps = [psum.tile([128, BLK], F32, tag=f"ps{h}", name=f"ps{h}") for h in range(NSPLIT)]

    def blk(t, h):
        return t[:, h * BLK:(h + 1) * BLK]

    def blkb(t, h):  # bottom half of a block
        return t[HF:, h * BLK:(h + 1) * BLK]

    def blkt(t, h):  # top half of a block
        return t[:HF, h * BLK:(h + 1) * BLK]

    def recip_fast_f16(out, in_):
        # reciprocal_approx_fast with an fp16 output AP: the BITWISE_NOT seed
        # only needs the fp32 INPUT bit layout; the write-side converts.
        from concourse.dve_ops import RECIP_APPROX_FAST_CONSTS, RECIPROCAL_APPROX_FAST
        c = RECIP_APPROX_FAST_CONSTS
        return nc.vector._custom_dve(
            RECIPROCAL_APPROX_FAST, out=out, in0=in_,
            s0=c["s0"], s1=c["s1"], imm2=c["imm2"],
        )

    import contextlib

    with nc.allow_low_precision(reason="fp16 feature chain; validated vs fp64 host sim"):
        for h in range(NSPLIT):
            # h0's chain outranks h1 in the scheduler so engine queues don't
            # stall behind not-yet-ready h1 ops
            with tc.high_priority() if h == 0 else contextlib.nullcontext():
                # chunk0 matmul only needs xy + weights; runs during the chain
                nc.tensor.matmul(ps[h][:], wt[:, 0:NBINS], xyb(h), start=True, stop=False)

                # squares: h0's x^2 on V (spine head), h1's on Pool
                if h == 0:
                    nc.vector.tensor_tensor(blk(sq, h), xxb(h), xxb(h), AluOpType.mult)
                else:
                    nc.gpsimd.tensor_tensor(blk(sq, h), xxb(h), xxb(h), AluOpType.mult)
                if h == 0:
                    nc.scalar.square(blk(syb, h), yyb(h))
                else:
                    nc.gpsimd.tensor_tensor(blk(syb, h), yyb(h), yyb(h), AluOpType.mult)
                nc.vector.tensor_tensor(blk(r2, h), blk(sq, h), blk(syb, h), AluOpType.add)
                nc.scalar.activation(blk(rf32, h), blk(r2, h), AFT.Sqrt, bias=eps[:])
                recip_fast_f16(blk(ir16, h), blk(rf32, h))
                # chunk2: just the r feature on top; the bottom 64 contraction
                # rows are dropped entirely (sin series truncated at m<=2), so
                # this is a 64-partition matmul ready straight off the sqrt,
                # emitted mid-group so the stop lands on the late w1 matmul.
                nc.scalar.activation(blkt(w2, h), blkt(r2, h), AFT.Sqrt, bias=eps[:HF])
                nc.tensor.matmul(ps[h][:], wt[:HF, 2 * NBINS:3 * NBINS],
                                 w2[:HF, h * BLK:(h + 1) * BLK], start=False, stop=False)
                # xyx = [x^2; xy] is input-only; scheduled into early V gaps.
                # w1 = xyx*(1/r) = [xu; yu] comes straight off the recip.
                nc.vector.tensor_tensor(blk(xyx, h), xyb(h), xxb(h), AluOpType.mult)
                nc.vector.tensor_tensor(blk(w1, h), blk(xyx, h), blk(ir16, h), AluOpType.mult)
                nc.tensor.matmul(ps[h][:], wt[:, NBINS:2 * NBINS], blk(w1, h), start=False, stop=True)

                # evict PSUM -> SBUF fp16 (bias added on host), then DMA out
                nc.scalar.copy(blk(outt, h), ps[h][:])
                nc.scalar.dma_start(out_d[:, h * BLK:(h + 1) * BLK], blk(outt, h))


_COMPILED = None


def _get_compiled():
    global _COMPILED
    if _COMPILED is None:
        nc = bacc.Bacc("TRN2", target_bir_lowering=False, debug=False)
        pk = nc.dram_tensor("pk", [128, 3 * KPC], F16, kind="ExternalInput").ap()
        w = nc.dram_tensor("w", [128, NCHUNK * NBINS], F16, kind="ExternalInput").ap()
        out = nc.dram_tensor("out", [NBINS, KPC], F16, kind="ExternalOutput").ap()
        with tile.TileContext(nc) as tc:
            _device_kernel(tc, out, pk, w)
        nc.compile()
        _COMPILED = nc
    return _COMPILED


# ----------------------------------------------------------------------------
# entry point
# ----------------------------------------------------------------------------

def _run(K, reference_angles, mu, kappa, weight, bias, **spmd_kwargs):
    K = np.ascontiguousarray(np.asarray(K, np.float32))
    x = K[:, 0::2].astype(np.float16)  # (NKEYS, NFREQ) real parts
    y = K[:, 1::2].astype(np.float16)  # imag parts

    W = _build_device_weights(reference_angles, mu, kappa, weight)
    in_maps = []
    for c in range(NCORES):
        sl = slice(c * KPC, (c + 1) * KPC)
        xt = np.ascontiguousarray(x[sl].T)  # (64, KPC)
        yt = np.ascontiguousarray(y[sl].T)
        # pk packs [yy-h | xx-h | xy-h] per key half (replicated x/y halves)
        pk = np.empty((128, 3 * KPC), np.float16)
        for h in range(NSPLIT):
            ksl = slice(h * BLK, (h + 1) * BLK)
            base = h * 3 * BLK
            pk[:NFREQ, base:base + BLK] = yt[:, ksl]
            pk[NFREQ:, base:base + BLK] = yt[:, ksl]
            pk[:NFREQ, base + BLK:base + 2 * BLK] = xt[:, ksl]
            pk[NFREQ:, base + BLK:base + 2 * BLK] = xt[:, ksl]
            pk[:NFREQ, base + 2 * BLK:base + 3 * BLK] = xt[:, ksl]
            pk[NFREQ:, base + 2 * BLK:base + 3 * BLK] = yt[:, ksl]
        in_maps.append({"pk": pk, "w": W})

    nc = _get_compiled()
    res = run_bass_kernel_spmd(nc, in_maps, list(range(NCORES)), **spmd_kwargs)

    bias32 = np.asarray(bias, np.float32)
    out = np.empty((NKEYS, NBINS), np.float32)
    for c in range(NCORES):
        out[c * KPC:(c + 1) * KPC] = res.results[c]["out"].T.astype(np.float32)
    out += bias32[None, :]
    return out, res


def kernel(K, reference_angles, mu, kappa, weight, bias):
    out, _ = _run(K, reference_angles, mu, kappa, weight, bias)
    return out
